# revision 33
# baseline (speedup 1.0000x reference)
"""GQA attention kernel for 8 Trainium2 NeuronCores.

Sharding: tensor-parallel over heads. Core i handles query heads (2i, 2i+1)
and KV head i//2. Out-proj is row-parallel: each core emits a partial
[S, DIM] output (bf16); the host sums the 8 partials and adds the bias.

Performance structure:
- everything staged/computed in bf16 (fp32 PSUM accumulation)
- software-pipelined chunk loop: attn(sc) -> proj(sc+1) -> out-proj(sc),
  so PE never waits on the DVE RoPE chain (overlaps out-proj) nor on the
  softmax-normalize chain (overlaps next proj)
- DMA split across both hwdge queues (SP + Activation engines); inputs
  consolidated into one DRAM tensor (2 DMAs per half-chunk slab)
- 1/den = exp(-ln(den)) on the Act engine (ln/exp share one act table);
  denominator via ones-matmul keeps everything partition-replicated
"""

import numpy as np
import ml_dtypes

DIM = 2048
Q_HEADS = 16
KV_HEADS = 4
HEAD_DIM = 128
S = 2048
MAX_LEN = 2048
ROPE_THETA = 10000.0
ROPE_FACTOR = 8.0
N_CORES = 8
SCALE = 1.0 / np.sqrt(HEAD_DIM)
NEG = -1.0e30
CS = 512          # query chunk size
NC_CH = S // CS   # 4 chunks

_F32R_CACHE = {}

BF = ml_dtypes.bfloat16


def _rope_cos_sin_T():
    d = HEAD_DIM
    seq_eff = max(S, MAX_LEN)
    base_adj = (ROPE_FACTOR * seq_eff / MAX_LEN - (ROPE_FACTOR - 1.0)) ** (d / (d - 2))
    adjusted_base = ROPE_THETA * base_adj
    inv_freq = 1.0 / adjusted_base ** (np.arange(0, d, 2, dtype=np.float32) / d)
    pos = np.arange(S, dtype=np.float32)
    freqs = pos[:, None] * inv_freq[None, :]
    emb = np.concatenate([freqs, freqs], axis=-1)  # [S, d]
    return (
        np.ascontiguousarray(np.cos(emb).T.astype(np.float32)),  # [d, S]
        np.ascontiguousarray(np.sin(emb).T.astype(np.float32)),
    )


def _masks():
    # additive masks for the 4 diagonal 128x512 blocks: block r covers keys
    # [128r, 128r+128) against queries [0, 512) within a 512-query chunk.
    k = np.arange(128)[:, None]
    q = np.arange(512)[None, :]
    m = np.zeros((128, 4, 512), np.float32)
    for r in range(4):
        m[:, r, :] = np.where(128 * r + k > q, NEG, 0.0).astype(np.float32)
    return np.ascontiguousarray(m.reshape(128, 4 * 512))


def _build_program():
    import concourse.bass as bass
    import concourse.tile as tile
    from concourse import mybir
    import bass_rust
    from concourse.vector_clock import ScopedClock
    from concourse.masks import make_identity

    # --- workaround: walrus CTRL instructions accept a single sync wait;
    # split the TileContext end-drain waits across one SP nop each.
    def _patched_drain_and_barrier(self, tick_clock, wait_clock):
        nop0 = self.nc.sync.nop(nofuse=True)
        wait_clock.add_sem_waits(nop0.ins, ScopedClock({None: tick_clock.global_clock}))
        si = nop0.ins.sync_info
        ws = list(si.on_wait) if si is not None else []
        if len(ws) > 1:
            nop0.ins.sync_info = bass_rust.SyncInfo(
                on_wait=ws[:1], on_update=list(si.on_update))
            for i in range(1, len(ws)):
                nop = self.nc.sync.nop(nofuse=True)
                nop.ins.sync_info = bass_rust.SyncInfo(on_wait=ws[i:i + 1], on_update=[])
        self.nc.sync.drain()
        self.nc.all_engine_barrier()
        popped = self.nc._tile_sem_poison_stack.pop()
        assert popped is self._sem_poison
        self.nc.clear_and_free_semaphores(list(self.sems.allocated().values()))
        self.nc.all_engine_barrier()

    tile.TileContext._drain_and_barrier = _patched_drain_and_barrier

    def _split_multi_waits(nc):
        # this walrus build accepts a single sync-wait slot on several
        # instruction encodings; peel extra waits onto same-engine NoOps.
        cnt = 0
        for f in nc.m.functions:
            for bb in f.blocks:
                new_l = []
                for inst in bb.instructions:
                    si = inst.sync_info
                    ws = list(si.on_wait) if si is not None else []
                    if len(ws) > 1:
                        for w in ws[:-1]:
                            nop = mybir.InstNoOp(
                                name=f"{inst.name}_wsplit{cnt}", engine=inst.engine,
                                bass_nofuse=True,
                                sync_info=mybir.SyncInfo(on_wait=[w], on_update=[]))
                            nc.register_instruction(nop, overwrite=True)
                            new_l.append(nop)
                            cnt += 1
                        inst.sync_info = mybir.SyncInfo(
                            on_wait=[ws[-1]], on_update=list(si.on_update))
                    new_l.append(inst)
                bb.instructions = new_l

    f32 = mybir.dt.float32
    bf16 = mybir.dt.bfloat16
    AF = mybir.ActivationFunctionType
    OP = mybir.AluOpType

    nc = bass.Bass()
    # inputs: [ci(128 part), sc(4), half(2), qkv(3), c8(8), s'(512)] bf16
    qkv_in = nc.dram_tensor("qkv_st", [128, NC_CH, 2, 3, 8, CS], bf16,
                            kind="ExternalInput")
    # weights: q-proj and kv-proj separate so each DMA is fully contiguous
    wqa_in = nc.dram_tensor("wqa_st", [128, 16, 256], bf16, kind="ExternalInput")
    wkv_in = nc.dram_tensor("wkv_st", [128, 16, 256], bf16, kind="ExternalInput")
    wo_in = nc.dram_tensor("wo_st", [128, 2, DIM], bf16, kind="ExternalInput")
    b_in = nc.dram_tensor("b_st", [128, 4], f32, kind="ExternalInput")
    # trig/mask: [ci, {cos,sin,mask}, 2048] bf16
    tm_in = nc.dram_tensor("tm_st", [128, 3, S], bf16, kind="ExternalInput")
    out_dram = nc.dram_tensor("partial", [S, DIM], bf16, kind="ExternalOutput")

    with tile.TileContext(nc) as tc:
        with (
            tc.tile_pool(name="const", bufs=1) as cpool,
            tc.tile_pool(name="slab", bufs=3) as spool,
            tc.tile_pool(name="attn", bufs=2) as atpool,
            tc.tile_pool(name="acts", bufs=1) as apool,
            tc.tile_pool(name="work", bufs=2) as wpool,
            tc.tile_pool(name="ot", bufs=4) as otpool,
            tc.tile_pool(name="psp", bufs=1, space="PSUM") as ps_proj,
            tc.tile_pool(name="pss", bufs=2, space="PSUM") as ps_attn,
            tc.tile_pool(name="psa", bufs=1, space="PSUM") as ps_acc,
        ):
            slab_q = {}
            _pending = [(sc, hh) for sc in range(NC_CH) for hh in range(2)]

            def load_half():
                # hh=0 halves on the sync queue, hh=1 on scalar: balances the
                # two queues and keeps at most one slab load ahead of any
                # output write (FIFO head-of-line blocking otherwise stalls
                # the out-proj copies -> psum WAR -> PE). q part separate so
                # the q-loop starts while k/v stream.
                sc, hh = _pending.pop(0)
                eng = nc.sync if hh == 0 else nc.scalar
                t = spool.tile([128, 3, 8, CS], bf16, tag="slab", name="slab")
                eng.dma_start(t[:, 0], qkv_in[:, sc, hh, 0])
                eng.dma_start(t[:, 1:3], qkv_in[:, sc, hh, 1:3])
                slab_q[(sc, hh)] = t

            # ---- preload. sync: wq, slab(0,0), wkv, slab(1,0).
            # scalar: biases, slab(0,1), trig, wo.
            wqa_sb = cpool.tile([128, 16, 256], bf16)
            nc.sync.dma_start(wqa_sb[:], wqa_in[:])
            b_sb = cpool.tile([128, 4], f32)
            nc.scalar.dma_start(b_sb[:], b_in[:])
            load_half()            # (0,0) -> buf A, sync
            load_half()            # (0,1) -> buf B, scalar (q, then kv)
            tm_sb = cpool.tile([128, 3, S], bf16)
            nc.scalar.dma_start(tm_sb[:], tm_in[:])
            wkv_sb = cpool.tile([128, 16, 256], bf16)
            nc.sync.dma_start(wkv_sb[:], wkv_in[:])
            load_half()            # (1,0) -> buf C, sync
            wo_sb = cpool.tile([128, 2, DIM], bf16)
            nc.scalar.dma_start(wo_sb[:], wo_in[:])

            ones_f = cpool.tile([128, 128], f32)
            nc.vector.memset(ones_f[:], 1.0)
            ones_mat = cpool.tile([128, 128], bf16)
            nc.vector.tensor_copy(out=ones_mat[:], in_=ones_f[:])
            ident_f = cpool.tile([128, 128], f32)
            make_identity(nc, ident_f[:])
            ident = cpool.tile([128, 128], bf16)
            nc.vector.tensor_copy(out=ident[:], in_=ident_f[:])

            # ---- persistent activations (full sequence)
            k_rot = apool.tile([128, S], bf16, tag="krot")
            v_sb = apool.tile([128, S], bf16, tag="vsb")
            # chunk-local
            q_rot = [apool.tile([128, CS], bf16, tag=f"qrot{h}", name=f"qrot{h}")
                     for h in range(2)]
            ctxT = [apool.tile([128, CS], bf16, tag=f"ctx{h}", name=f"ctx{h}")
                    for h in range(2)]

            def rope(dst, raw, sc):
                # dst = raw*cos + swap(raw)*sinMod; sinMod has -1 baked into
                # the low half host-side (rotate_half sign).
                ssl = slice(sc * CS, sc * CS + CS)
                swp = wpool.tile([128, CS], bf16, tag="ropeswp")
                nc.vector.tensor_copy(out=swp[0:64, :], in_=raw[64:128, :])
                nc.vector.tensor_copy(out=swp[64:128, :], in_=raw[0:64, :])
                tmp = wpool.tile([128, CS], bf16, tag="ropetmp")
                nc.vector.tensor_tensor(tmp[:], swp[:], tm_sb[:, 1, ssl], OP.mult)
                nc.vector.tensor_tensor(dst, raw[:], tm_sb[:, 0, ssl], OP.mult)
                nc.vector.tensor_tensor(dst, dst, tmp[:], OP.add)

            def do_proj(sc):
                ssl = slice(sc * CS, sc * CS + CS)
                pq0 = ps_proj.tile([128, CS], f32, tag="pq0")
                pq1 = ps_proj.tile([128, CS], f32, tag="pq1")
                pk = ps_proj.tile([128, CS], f32, tag="pk")
                pv = ps_proj.tile([128, CS], f32, tag="pv")
                slabs = [slab_q.pop((sc, 0)), slab_q.pop((sc, 1))]
                # per-tensor loops: q compute starts while k/v parts stream
                for hh in range(2):
                    for c8 in range(8):
                        cc = 8 * hh + c8
                        st_, sp_ = cc == 0, cc == 15
                        nc.tensor.matmul(pq0[:], wqa_sb[:, cc, 0:128],
                                         slabs[hh][:, 0, c8], start=st_, stop=sp_)
                        nc.tensor.matmul(pq1[:], wqa_sb[:, cc, 128:256],
                                         slabs[hh][:, 0, c8], start=st_, stop=sp_)
                for hh in range(2):
                    for c8 in range(8):
                        cc = 8 * hh + c8
                        nc.tensor.matmul(pk[:], wkv_sb[:, cc, 0:128],
                                         slabs[hh][:, 1, c8],
                                         start=cc == 0, stop=cc == 15)
                for hh in range(2):
                    for c8 in range(8):
                        cc = 8 * hh + c8
                        nc.tensor.matmul(pv[:], wkv_sb[:, cc, 128:256],
                                         slabs[hh][:, 2, c8],
                                         start=cc == 0, stop=cc == 15)
                # prefetch: keep 3 half-slabs in flight (bufs=3 rotation —
                # each new load WARs only on reads finished 3 halves ago).
                # proj(0) issues 1 ((1,1)); proj(1..) issue 2.
                n_pref = 1 if sc == 0 else 2
                for _ in range(n_pref):
                    if _pending:
                        load_half()

                # bias + RoPE; k first (both heads' scores need it)
                k_raw = wpool.tile([128, CS], bf16, tag="raw")
                nc.scalar.activation(k_raw[:], pk[:], AF.Identity, bias=b_sb[:, 2:3])
                rope(k_rot[:, ssl], k_raw, sc)
                q0_raw = wpool.tile([128, CS], bf16, tag="raw")
                nc.scalar.activation(q0_raw[:], pq0[:], AF.Identity, bias=b_sb[:, 0:1])
                rope(q_rot[0][:], q0_raw, sc)
                q1_raw = wpool.tile([128, CS], bf16, tag="raw")
                nc.scalar.activation(q1_raw[:], pq1[:], AF.Identity, bias=b_sb[:, 1:2])
                rope(q_rot[1][:], q1_raw, sc)
                v_raw = wpool.tile([128, CS], bf16, tag="raw")
                nc.scalar.activation(v_raw[:], pv[:], AF.Identity, bias=b_sb[:, 3:4])
                for j, vtag in enumerate(("pq0", "pq1", "pk", "pv")):
                    ptr = ps_proj.tile([128, 128], bf16, tag=vtag, name="ptr")
                    nc.tensor.transpose(ptr[:], v_raw[:, j * 128:(j + 1) * 128],
                                        ident[:])
                    nc.vector.tensor_copy(
                        out=v_sb[:, (sc * 4 + j) * 128:(sc * 4 + j) * 128 + 128],
                        in_=ptr[:])

            def do_attn(sc):
                n_kt = 4 * (sc + 1)
                for h in range(2):
                    attnT = atpool.tile([128, 16, CS], bf16, tag="attnT",
                                        name="attnT")
                    for kt in range(n_kt):
                        pst = ps_attn.tile([128, CS], f32, tag="sT", name="pst")
                        nc.tensor.matmul(
                            pst[:], k_rot[:, kt * 128:(kt + 1) * 128],
                            q_rot[h][:], start=True, stop=True)
                        r = kt - 4 * sc
                        if r >= 0:
                            nc.vector.tensor_tensor(
                                pst[:], pst[:],
                                tm_sb[:, 2, r * CS:(r + 1) * CS], OP.add)
                        nc.scalar.activation(attnT[:, kt], pst[:], AF.Exp,
                                             scale=float(SCALE))
                    den = ps_acc.tile([128, CS], f32, tag="den")
                    pctx = ps_acc.tile([128, CS], f32, tag="ctx")
                    for kt in range(n_kt):
                        nc.tensor.matmul(den[:], ones_mat[:], attnT[:, kt],
                                         start=kt == 0, stop=kt == n_kt - 1)
                    for kt in range(n_kt):
                        nc.tensor.matmul(pctx[:], v_sb[:, kt * 128:(kt + 1) * 128],
                                         attnT[:, kt],
                                         start=kt == 0, stop=kt == n_kt - 1)
                    # 1/den = exp(-ln(den)) on Act (ln/exp share a table)
                    den_ln = wpool.tile([128, CS], f32, tag="dln")
                    nc.scalar.activation(den_ln[:], den[:], AF.Ln)
                    den_inv = wpool.tile([128, CS], bf16, tag="dinv")
                    nc.scalar.activation(den_inv[:], den_ln[:], AF.Exp, scale=-1.0)
                    nc.vector.tensor_tensor(ctxT[h][:], pctx[:], den_inv[:], OP.mult)

            def do_outproj(sc):
                for st in range(4):
                    rows = slice(sc * CS + st * 128, sc * CS + st * 128 + 128)
                    # po on the attn banks (free after attn(sc); proj(sc+1)
                    # follows immediately in plain order so proj banks would
                    # couple its matmuls to out-proj copies)
                    po = [ps_attn.tile([128, CS], f32, tag="sT", name="po0"),
                          ps_attn.tile([128, CS], f32, tag="sT", name="po1"),
                          ps_acc.tile([128, CS], f32, tag="den", name="po2"),
                          ps_acc.tile([128, CS], f32, tag="ctx", name="po3")]
                    for h in range(2):
                        for ec in range(4):
                            esl = slice(ec * CS, ec * CS + CS)
                            nc.tensor.matmul(po[ec][:],
                                             ctxT[h][:, st * 128:st * 128 + 128],
                                             wo_sb[:, h, esl],
                                             start=h == 0, stop=h == 1)
                    ot = otpool.tile([128, DIM], bf16, tag="ot", name="ot")
                    for ec in range(4):
                        esl = slice(ec * CS, ec * CS + CS)
                        if ec % 2 == 0:
                            nc.vector.tensor_copy(out=ot[:, esl], in_=po[ec][:])
                        else:
                            nc.scalar.activation(ot[:, esl], po[ec][:], AF.Copy)
                    nc.scalar.dma_start(out_dram[rows, :], ot[:])

            # ---- chunk loop, plain order (measured faster than the
            # attn->proj(sc+1)->outproj software pipeline: the DMA startup
            # transient dominates the early chunks either way, and the
            # plain order couples fewer psum WARs across phases)
            for sc in range(NC_CH):
                do_proj(sc)
                do_attn(sc)
                do_outproj(sc)

    _split_multi_waits(nc)
    return nc


def _stage_qkv(q2, k2, v2):
    # each [S, DIM] f32 -> combined [ci, sc, half, qkv, c8, s'] bf16
    def part(x):
        # [S, DIM] -> [ci, sc, co, s'] -> [ci, sc, half, c8, s']
        a = x.reshape(NC_CH, CS, 16, 128).transpose(3, 0, 2, 1)
        return a.reshape(128, NC_CH, 2, 8, CS)
    stacked = np.stack([part(q2), part(k2), part(v2)], axis=3)
    return np.ascontiguousarray(stacked.astype(BF))


def kernel(query, key, value, Wq, bq, Wk, bk, Wv, bv, Wo, bo):
    from concourse.bass_utils import run_bass_kernel_spmd

    query = np.asarray(query, np.float32)
    B = query.shape[0]
    qkv_st = _stage_qkv(query.reshape(S, DIM),
                        np.asarray(key, np.float32).reshape(S, DIM),
                        np.asarray(value, np.float32).reshape(S, DIM))
    cosT, sinT = _rope_cos_sin_T()
    sinT = sinT.copy()
    sinT[0:64, :] *= -1.0  # rotate_half: low half gets -x2*sin
    tm_st = np.ascontiguousarray(
        np.stack([cosT, sinT, _masks()], axis=1).astype(BF))  # [128, 3, S]

    if "nc" not in _F32R_CACHE:
        _F32R_CACHE["nc"] = _build_program()
    nc = _F32R_CACHE["nc"]

    Wq_f = np.asarray(Wq, np.float32)
    Wk_f = np.asarray(Wk, np.float32)
    Wv_f = np.asarray(Wv, np.float32)
    Wo_f = np.asarray(Wo, np.float32)
    bq_f = np.asarray(bq, np.float32)
    bk_f = np.asarray(bk, np.float32)
    bv_f = np.asarray(bv, np.float32)

    in_maps = []
    for i in range(N_CORES):
        g = i // 2
        wq_st = (Wq_f[256 * i:256 * (i + 1), :].T.reshape(16, 128, 256)
                 .transpose(1, 0, 2))
        wk_st = (Wk_f[128 * g:128 * (g + 1), :].T.reshape(16, 128, 128)
                 .transpose(1, 0, 2))
        wv_st = (Wv_f[128 * g:128 * (g + 1), :].T.reshape(16, 128, 128)
                 .transpose(1, 0, 2))
        wqa_s = np.ascontiguousarray(wq_st.astype(BF))
        wkv_s = np.ascontiguousarray(
            np.concatenate([wk_st, wv_st], axis=2).astype(BF))
        wo_st = np.ascontiguousarray(
            Wo_f[:, 256 * i:256 * (i + 1)].T.reshape(2, 128, DIM)
            .transpose(1, 0, 2).astype(BF))
        b_st = np.ascontiguousarray(np.stack([
            bq_f[256 * i:256 * i + 128],
            bq_f[256 * i + 128:256 * i + 256],
            bk_f[128 * g:128 * (g + 1)],
            bv_f[128 * g:128 * (g + 1)],
        ], axis=1))  # [128, 4] f32
        in_maps.append({
            "qkv_st": qkv_st, "wqa_st": wqa_s, "wkv_st": wkv_s,
            "wo_st": wo_st, "b_st": b_st, "tm_st": tm_st,
        })

    _F32R_CACHE["in_maps"] = in_maps
    globals()["_LAST_IN_MAPS"] = in_maps
    res = run_bass_kernel_spmd(nc, in_maps, list(range(N_CORES)))
    out = res.results[0]["partial"].astype(np.float32)
    for i in range(1, N_CORES):
        out = out + res.results[i]["partial"].astype(np.float32)
    out = out + np.asarray(bo, np.float32)[None, :]
    return out.reshape(B, S, DIM).astype(np.float32)
